# revision 4
# baseline (speedup 1.0000x reference)
"""Trainium2 Bass kernel for a causal single-head attention block.

Reference computation (fp32):
    q = x @ Wq; k = x @ Wk; v = x @ Wv        x: [B=256, T=256, C=384], W*: [384, 64]
    wei = softmax(causal_mask(q @ k.T / sqrt(C)))
    out = wei @ v                              out: [256, 256, 64]

Strategy: pure data parallel over B across 8 NeuronCores (32 batches/core).
Per batch on-device pipeline (everything M=128-packed where possible):
    xT   = PE-transpose(x_b)                      [C, T] layout, 3 chunks of [128, 256]
    qT   = Wq.T @ xT   (3 accumulating matmuls)   [64, 256]   (fp32r, N=256 -> 1 cyc/row)
    kT   = Wk.T @ xT                              [64, 256]
    v    = xT.T @ Wv   (6 matmuls)                [128, 64] x2 (t tiles)
    sT   = kT[:, s_tile].T @ qT                   scores transposed [s, t], 2 tiles [128, 256]
    P    = exp((sT + mask_bias) * 1/sqrt(C))      mask_bias = -1e9 where s > t (causal)
    oeT  = [v | 1].T @ P  (accumulate s tiles)    [65, 256]; row 64 = softmax denominators Z
    out  = PE-transpose(oeT) * (1/Z)              [128, 64] x2 -> DMA
"""

import numpy as np

N_EMBED = 384
HEAD_SIZE = 64
T = 256
B = 256
N_CORES = 8
B_SHARD = B // N_CORES  # 32
CC = N_EMBED // 128  # 3 contraction chunks
INV_SQRT_C = 1.0 / float(np.sqrt(N_EMBED))
MASK_NEG = -1.0e9

_CACHE = {}

# test.py can flip these before calling kernel()
TRACE = False
LAST_RESULTS = None
USE_FP32R = True


def _build_program():
    import concourse.bacc as bacc
    import concourse.mybir as mybir
    import concourse.tile as tile
    from concourse import bass

    f32 = mybir.dt.float32
    # dtype for matmul operands: fp32r (TF32-like, 4x matmul throughput at
    # free dim >= 256) or full fp32
    mdt = mybir.dt.float32r if USE_FP32R else f32
    ts = bass.ts

    nc = bacc.Bacc("TRN2", target_bir_lowering=False, debug=False,
                   enable_asserts=False)

    x_d = nc.dram_tensor("x", [B_SHARD, T, N_EMBED], f32, kind="ExternalInput")
    wq_d = nc.dram_tensor("Wq", [N_EMBED, HEAD_SIZE], f32, kind="ExternalInput")
    wk_d = nc.dram_tensor("Wk", [N_EMBED, HEAD_SIZE], f32, kind="ExternalInput")
    wv_d = nc.dram_tensor("Wv", [N_EMBED, HEAD_SIZE], f32, kind="ExternalInput")
    ident_d = nc.dram_tensor("ident", [128, 128], f32, kind="ExternalInput")
    bias_d = nc.dram_tensor("maskbias", [2, 128, T], f32, kind="ExternalInput")
    out_d = nc.dram_tensor("out", [B_SHARD, T, HEAD_SIZE], f32, kind="ExternalOutput")

    x_ap = x_d.ap()
    out_ap = out_d.ap()

    with tile.TileContext(nc) as tc:
        with (
            tc.tile_pool(name="const", bufs=1) as cpool,
            tc.tile_pool(name="xin", bufs=4) as xin_pool,
            tc.tile_pool(name="xt", bufs=6) as xt_pool,
            tc.tile_pool(name="proj", bufs=10) as proj_pool,
            tc.tile_pool(name="soft", bufs=8) as soft_pool,
            tc.tile_pool(name="outp", bufs=10) as out_pool,
            tc.tile_pool(name="ps", bufs=7, space="PSUM") as ps_pool,
        ):
            # ---- constants: weights (3 chunks each), identity, mask biases ----
            ident = cpool.tile([128, 128], f32, tag="ident")
            nc.sync.dma_start(ident[:], ident_d.ap())
            wq_sb, wk_sb, wv_sb = [], [], []
            for cc in range(CC):
                for name, dram, lst in (("wq", wq_d, wq_sb), ("wk", wk_d, wk_sb),
                                        ("wv", wv_d, wv_sb)):
                    stage = cpool.tile([128, HEAD_SIZE], f32, tag=f"{name}s{cc}")
                    nc.sync.dma_start(stage[:], dram.ap()[ts(cc, 128), :])
                    t_ = cpool.tile([128, HEAD_SIZE], mdt, tag=f"{name}{cc}")
                    nc.vector.tensor_copy(t_[:], stage[:])
                    lst.append(t_)
            bias_sb = []
            for st in range(2):
                t_ = cpool.tile([128, T], f32, tag=f"bias{st}")
                nc.sync.dma_start(t_[:], bias_d.ap()[st, :, :])
                bias_sb.append(t_)
            ones_col = cpool.tile([128, 1], f32, tag="ones")
            nc.gpsimd.memset(ones_col[:], 1.0)

            for b in range(B_SHARD):
                # ---- load x_b in natural [T, C] layout ----
                x_nat = []
                for tt in range(2):
                    t_ = xin_pool.tile([128, N_EMBED], f32, tag="x_nat")
                    nc.sync.dma_start(t_[:], x_ap[b, ts(tt, 128), :])
                    x_nat.append(t_)

                # ---- xT: [C, T] layout, 3 chunks of [128, 256] ----
                xt_sb = []
                for cc in range(CC):
                    ps = ps_pool.tile([128, T], f32, tag="ps")
                    for tt in range(2):
                        nc.tensor.transpose(ps[:, ts(tt, 128)],
                                            x_nat[tt][:, ts(cc, 128)], ident[:])
                    sb = xt_pool.tile([128, T], mdt, tag="xt")
                    nc.vector.tensor_copy(sb[:], ps[:])
                    xt_sb.append(sb)

                # ---- qT, kT: [64, 256] ----
                qkT = []
                for w_sb, nm in ((wq_sb, "q"), (wk_sb, "k")):
                    ps = ps_pool.tile([128, T], f32, tag="ps")
                    for cc in range(CC):
                        nc.tensor.matmul(ps[:HEAD_SIZE, :], w_sb[cc][:],
                                         xt_sb[cc][:],
                                         start=(cc == 0), stop=(cc == CC - 1))
                    sb = proj_pool.tile([HEAD_SIZE, T], mdt, tag=f"{nm}T")
                    nc.vector.tensor_copy(sb[:], ps[:HEAD_SIZE, :])
                    qkT.append(sb)
                qT_sb, kT_sb = qkT

                # ---- v with ones column: [128, 65] per t tile ----
                v_ext = []
                for tt in range(2):
                    ps = ps_pool.tile([128, T], f32, tag="ps")
                    for cc in range(CC):
                        nc.tensor.matmul(ps[:, :HEAD_SIZE],
                                         xt_sb[cc][:, ts(tt, 128)], wv_sb[cc][:],
                                         start=(cc == 0), stop=(cc == CC - 1))
                    sb = proj_pool.tile([128, HEAD_SIZE + 1], mdt, tag="v_ext")
                    nc.vector.tensor_copy(sb[:, :HEAD_SIZE], ps[:, :HEAD_SIZE])
                    nc.vector.tensor_copy(sb[:, HEAD_SIZE:HEAD_SIZE + 1], ones_col[:])
                    v_ext.append(sb)

                # ---- scores^T, mask, exp ----
                p_sb = []
                for st in range(2):
                    ps = ps_pool.tile([128, T], f32, tag="ps")
                    nc.tensor.matmul(ps[:], kT_sb[:, ts(st, 128)],
                                     qT_sb[:], start=True, stop=True)
                    msk = soft_pool.tile([128, T], f32, tag="msk")
                    nc.vector.tensor_add(msk[:], ps[:], bias_sb[st][:])
                    p = soft_pool.tile([128, T], mdt, tag="p")
                    nc.scalar.activation(p[:], msk[:],
                                         mybir.ActivationFunctionType.Exp,
                                         scale=INV_SQRT_C)
                    p_sb.append(p)

                # ---- out_ext^T = [v|1].T @ P : [65, 256] ----
                oe_ps = ps_pool.tile([128, T], f32, tag="ps")
                for st in range(2):
                    nc.tensor.matmul(oe_ps[:HEAD_SIZE + 1, :], v_ext[st][:],
                                     p_sb[st][:],
                                     start=(st == 0), stop=(st == 1))
                oe_sb = out_pool.tile([HEAD_SIZE + 1, T], f32, tag="oe")
                nc.vector.tensor_copy(oe_sb[:], oe_ps[:HEAD_SIZE + 1, :])

                # ---- transpose back, normalize by Z, store ----
                for tt in range(2):
                    ps = ps_pool.tile([128, T], f32, tag="ps")
                    nc.tensor.transpose(ps[:, :HEAD_SIZE + 1],
                                        oe_sb[:, ts(tt, 128)],
                                        ident[:HEAD_SIZE + 1, :HEAD_SIZE + 1])
                    rz = out_pool.tile([128, 1], f32, tag="rz")
                    nc.vector.reciprocal(rz[:], ps[:, HEAD_SIZE:HEAD_SIZE + 1])
                    o = out_pool.tile([128, HEAD_SIZE], f32, tag="o")
                    nc.vector.tensor_scalar_mul(o[:], ps[:, :HEAD_SIZE], rz[:])
                    nc.sync.dma_start(out_ap[b, ts(tt, 128), :], o[:])

    nc.compile()
    return nc


def _consts():
    ident = np.eye(128, dtype=np.float32)
    # maskbias[st][s_local, t] = MASK_NEG where global s > t (causal mask)
    s = np.arange(T)[:, None]
    t = np.arange(T)[None, :]
    full = np.where(s > t, np.float32(MASK_NEG), np.float32(0.0)).astype(np.float32)
    bias = np.stack([full[:128], full[128:]], axis=0)
    return ident, bias


def kernel(x, Wq, Wk, Wv):
    global LAST_RESULTS
    from concourse import bass_utils

    if "nc" not in _CACHE:
        _CACHE["nc"] = _build_program()
    nc = _CACHE["nc"]

    x = np.ascontiguousarray(x, dtype=np.float32)
    Wq = np.ascontiguousarray(Wq, dtype=np.float32)
    Wk = np.ascontiguousarray(Wk, dtype=np.float32)
    Wv = np.ascontiguousarray(Wv, dtype=np.float32)
    ident, bias = _consts()

    in_maps = []
    for c in range(N_CORES):
        in_maps.append({
            "x": x[c * B_SHARD:(c + 1) * B_SHARD],
            "Wq": Wq, "Wk": Wk, "Wv": Wv,
            "ident": ident, "maskbias": bias,
        })

    res = bass_utils.run_bass_kernel_spmd(
        nc, in_maps, core_ids=list(range(N_CORES)), trace=TRACE)
    LAST_RESULTS = res
    out = np.concatenate([res.results[c]["out"] for c in range(N_CORES)], axis=0)
    return out


# revision 9
# speedup vs baseline: 1.1746x; 1.1746x over previous
"""Trainium2 Bass kernel for a causal single-head attention block.

Reference computation (fp32):
    q = x @ Wq; k = x @ Wk; v = x @ Wv        x: [B=256, T=256, C=384], W*: [384, 64]
    wei = softmax(causal_mask(q @ k.T / sqrt(C)))
    out = wei @ v                              out: [256, 256, 64]

Strategy: pure data parallel over B across 8 NeuronCores (32 batches/core).

Per-batch pipeline (software-pipelined across batches so the PE never waits on
the softmax chain):
    xT   = PE-transpose(x_b)                   [C, T], 3 chunks [128, 256] (fp32r)
    qT|kT = W.T @ xT                           one [64, 512] psum (fp32r, N=256)
    v    = xT.T @ Wv                           [128, 64+64] (both t tiles, one psum)
    sT   = kT[:, s_tile].T @ qT                scores transposed [s, t], [128, 512] psum
    P    = exp((sT + tri_bias) * 1/sqrt(C))    tri mask only on diagonal 128x128 blocks;
                                               off-diagonal blocks exp'd straight from
                                               psum (never masked) or zeroed (always
                                               masked)
    oeT  = [v | 1].T @ P                       [65, 256]; row 64 = softmax denominator Z
    out  = PE-transpose(oeT) * (1/Z)           normalize on ScalarE, DMA out

Engine split: PE matmuls; DVE psum->sbuf operand copies + masks + reciprocal;
ScalarE (ACT) exp + v/ones copies + final normalize; fp32r (TF32-like) operands
everywhere for single-pass matmuls at 1 cyc/row (N>=256).
"""

import numpy as np

N_EMBED = 384
HEAD_SIZE = 64
H1 = HEAD_SIZE + 1
T = 256
B = 256
N_CORES = 8
B_SHARD = B // N_CORES  # 32
CC = N_EMBED // 128  # 3 contraction chunks
INV_SQRT_C = 1.0 / float(np.sqrt(N_EMBED))
MASK_NEG = -1.0e9

_CACHE = {}

# test.py can flip these before calling kernel()
TRACE = False
LAST_RESULTS = None
USE_FP32R = True


def _build_program():
    import concourse.bacc as bacc
    import concourse.mybir as mybir
    import concourse.tile as tile
    from concourse import bass

    f32 = mybir.dt.float32
    mdt = mybir.dt.float32r if USE_FP32R else f32
    ts = bass.ts
    Exp = mybir.ActivationFunctionType.Exp
    Copy = mybir.ActivationFunctionType.Copy

    nc = bacc.Bacc("TRN2", target_bir_lowering=False, debug=False,
                   enable_asserts=False)

    x_d = nc.dram_tensor("x", [B_SHARD, T, N_EMBED], f32, kind="ExternalInput")
    wq_d = nc.dram_tensor("Wq", [N_EMBED, HEAD_SIZE], f32, kind="ExternalInput")
    wk_d = nc.dram_tensor("Wk", [N_EMBED, HEAD_SIZE], f32, kind="ExternalInput")
    wv_d = nc.dram_tensor("Wv", [N_EMBED, HEAD_SIZE], f32, kind="ExternalInput")
    ident_d = nc.dram_tensor("ident", [128, 128], f32, kind="ExternalInput")
    tri_d = nc.dram_tensor("tribias", [2, 128, 128], f32, kind="ExternalInput")
    out_d = nc.dram_tensor("out", [B_SHARD, T, HEAD_SIZE], f32, kind="ExternalOutput")

    x_ap = x_d.ap().bitcast(mdt)
    out_ap = out_d.ap()

    with tile.TileContext(nc) as tc:
        with (
            tc.tile_pool(name="const", bufs=1) as cpool,
            tc.tile_pool(name="xin", bufs=6) as xin_pool,
            tc.tile_pool(name="xt", bufs=2) as xt_pool,
            tc.tile_pool(name="proj", bufs=4) as proj_pool,
            tc.tile_pool(name="soft", bufs=6) as soft_pool,
            tc.tile_pool(name="outp", bufs=8) as out_pool,
            tc.tile_pool(name="ps", bufs=8, space="PSUM") as ps_pool,
        ):
            # ---- constants ----
            ident = cpool.tile([128, 128], mdt, tag="ident")
            nc.sync.dma_start(ident[:], ident_d.ap().bitcast(mdt))
            wq_sb, wk_sb, wv_sb = [], [], []
            for cc in range(CC):
                for name, dram, lst in (("wq", wq_d, wq_sb), ("wk", wk_d, wk_sb),
                                        ("wv", wv_d, wv_sb)):
                    stage = cpool.tile([128, HEAD_SIZE], f32, tag=f"{name}s{cc}")
                    nc.sync.dma_start(stage[:], dram.ap()[ts(cc, 128), :])
                    t_ = cpool.tile([128, HEAD_SIZE], mdt, tag=f"{name}{cc}")
                    nc.vector.tensor_copy(t_[:], stage[:])
                    lst.append(t_)
            tri_sb = []
            for st in range(2):
                t_ = cpool.tile([128, 128], f32, tag=f"tri{st}")
                nc.sync.dma_start(t_[:], tri_d.ap()[st, :, :])
                tri_sb.append(t_)
            ones_col = cpool.tile([128, 1], f32, tag="ones")
            nc.gpsimd.memset(ones_col[:], 1.0)

            def load_x(b):
                tiles = []
                for tt in range(2):
                    t_ = xin_pool.tile([128, N_EMBED], mdt, tag="x_nat")
                    nc.sync.dma_start(t_[:], x_ap[b, ts(tt, 128), :])
                    tiles.append(t_)
                return tiles

            def transpose_x(x_nat):
                """-> one [128, 768] sbuf tile; chunk cc at [:, cc*256:(cc+1)*256]"""
                psA = ps_pool.tile([128, 512], mdt, tag="ps")   # cc 0,1
                psB = ps_pool.tile([128, 512], mdt, tag="ps")   # cc 2
                for cc in range(CC):
                    p = psA if cc < 2 else psB
                    off = (cc % 2) * 256
                    for tt in range(2):
                        nc.tensor.transpose(p[:, off + tt * 128:off + (tt + 1) * 128],
                                            x_nat[tt][:, ts(cc, 128)], ident[:])
                sb = xt_pool.tile([128, 3 * T], mdt, tag="xt")
                nc.vector.tensor_copy(sb[:, :512], psA[:])
                nc.vector.tensor_copy(sb[:, 512:], psB[:, :256])
                return sb

            def qkT_mm(xt):
                ps = ps_pool.tile([128, 512], f32, tag="ps")
                for cc in range(CC):
                    xc = xt[:, ts(cc, T)]
                    nc.tensor.matmul(ps[:HEAD_SIZE, :T], wq_sb[cc][:], xc,
                                     start=(cc == 0), stop=(cc == CC - 1))
                for cc in range(CC):
                    xc = xt[:, ts(cc, T)]
                    nc.tensor.matmul(ps[:HEAD_SIZE, T:], wk_sb[cc][:], xc,
                                     start=(cc == 0), stop=(cc == CC - 1))
                sb = proj_pool.tile([HEAD_SIZE, 512], mdt, tag="qk")
                nc.vector.tensor_copy(sb[:], ps[:HEAD_SIZE, :])
                return sb  # qT = [:, :256], kT = [:, 256:]

            def v_mm(xt):
                ps = ps_pool.tile([128, 512], f32, tag="ps")
                for tt in range(2):
                    for cc in range(CC):
                        nc.tensor.matmul(ps[:, tt * 128:tt * 128 + HEAD_SIZE],
                                         xt[:, cc * T + tt * 128: cc * T + (tt + 1) * 128],
                                         wv_sb[cc][:],
                                         start=(cc == 0), stop=(cc == CC - 1))
                v_ext = []
                for tt in range(2):
                    sb = proj_pool.tile([128, H1], mdt, tag="v_ext")
                    nc.scalar.activation(sb[:, :HEAD_SIZE],
                                         ps[:, tt * 128:tt * 128 + HEAD_SIZE],
                                         Copy)
                    nc.scalar.activation(sb[:, HEAD_SIZE:H1], ones_col[:], Copy)
                    v_ext.append(sb)
                return v_ext

            def scores_mm(qk):
                ps = ps_pool.tile([128, 512], f32, tag="ps")
                for st in range(2):
                    nc.tensor.matmul(ps[:, ts(st, T)], qk[:, T + st * 128:T + (st + 1) * 128],
                                     qk[:, :T], start=True, stop=True)
                return ps  # scoresT: s_tile st at [:, st*256:(st+1)*256]

            def softmax(sc_ps):
                """P tiles [128, 256] fp32r per s_tile, causal-masked, unnormalized."""
                # s0: cols t<128 diagonal (tri mask), cols t>=128 never masked
                # s1: cols t<128 always masked (zero), cols t>=128 diagonal
                p0 = soft_pool.tile([128, T], mdt, tag="p0")
                p1 = soft_pool.tile([128, T], mdt, tag="p1")
                m0 = soft_pool.tile([128, 128], f32, tag="m0")
                m1 = soft_pool.tile([128, 128], f32, tag="m1")
                nc.vector.tensor_add(m0[:], sc_ps[:, 0:128], tri_sb[0][:])
                nc.vector.tensor_add(m1[:], sc_ps[:, 384:512], tri_sb[1][:])
                nc.scalar.activation(p0[:, 0:128], m0[:], Exp, scale=INV_SQRT_C)
                nc.scalar.activation(p0[:, 128:256], sc_ps[:, 128:256], Exp,
                                     scale=INV_SQRT_C)
                nc.scalar.activation(p1[:, 0:128], ident[:].bitcast(f32), Copy,
                                     scale=0.0)
                nc.scalar.activation(p1[:, 128:256], m1[:], Exp, scale=INV_SQRT_C)
                return [p0, p1]

            def oe_mm(v_ext, p_sb):
                ps = ps_pool.tile([128, 512], f32, tag="ps")
                for st in range(2):
                    nc.tensor.matmul(ps[:H1, :T], v_ext[st][:], p_sb[st][:],
                                     start=(st == 0), stop=(st == 1))
                sb = out_pool.tile([H1 + 1, T], mdt, tag="oe")
                # row 65 is never written by the matmul and never read after the
                # transpose; copying 66 rows keeps partition bases aligned
                nc.vector.tensor_copy(sb[:], ps[:H1 + 1, :T])
                return sb

            def fin_mm(oe):
                ps = ps_pool.tile([128, 512], mdt, tag="ps")
                for tt in range(2):
                    nc.tensor.transpose(ps[:, tt * 128:tt * 128 + H1 + 1],
                                        oe[:, ts(tt, 128)], ident[:H1 + 1, :H1 + 1])
                return ps

            def norm_store(b, fin_ps):
                fps = fin_ps[:].bitcast(f32)
                for tt in range(2):
                    rz = out_pool.tile([128, 1], f32, tag="rz")
                    nc.vector.reciprocal(
                        rz[:], fps[:, tt * 128 + HEAD_SIZE: tt * 128 + H1])
                    o = out_pool.tile([128, HEAD_SIZE], f32, tag="o")
                    nc.scalar.activation(o[:], fps[:, tt * 128: tt * 128 + HEAD_SIZE],
                                         Copy, scale=rz[:])
                    nc.sync.dma_start(out_ap[b, ts(tt, 128), :], o[:])

            # ---- software-pipelined batch loop ----
            # PE stream per iteration: qkT(b) v(b) sc(b) T(b+1) oe(b) finT(b-1);
            # the DVE/ACT softmax chain of batch b runs under T(b+1), and
            # finT(b-1)'s oe-copy dependency is a full iteration old.
            x_nat = load_x(0)
            load_x_next = load_x(1)
            xt = transpose_x(x_nat)
            prev_oe = None
            for b in range(B_SHARD):
                qk = qkT_mm(xt)
                v_ext = v_mm(xt)
                sc_ps = scores_mm(qk)
                p_sb = softmax(sc_ps)
                if b + 1 < B_SHARD:
                    xt = transpose_x(load_x_next)
                if b + 2 < B_SHARD:
                    load_x_next = load_x(b + 2)
                oe_prev = prev_oe
                prev_oe = oe_mm(v_ext, p_sb)
                if oe_prev is not None:
                    norm_store(b - 1, fin_mm(oe_prev))
            norm_store(B_SHARD - 1, fin_mm(prev_oe))

    nc.compile()
    return nc


def _consts():
    ident = np.eye(128, dtype=np.float32)
    # tri bias for the two diagonal 128x128 blocks: MASK_NEG where s_local > t_local
    s = np.arange(128)[:, None]
    t = np.arange(128)[None, :]
    tri = np.where(s > t, np.float32(MASK_NEG), np.float32(0.0)).astype(np.float32)
    tribias = np.stack([tri, tri], axis=0)
    return ident, tribias


def kernel(x, Wq, Wk, Wv):
    global LAST_RESULTS
    from concourse import bass_utils

    if "nc" not in _CACHE:
        _CACHE["nc"] = _build_program()
    nc = _CACHE["nc"]

    x = np.ascontiguousarray(x, dtype=np.float32)
    Wq = np.ascontiguousarray(Wq, dtype=np.float32)
    Wk = np.ascontiguousarray(Wk, dtype=np.float32)
    Wv = np.ascontiguousarray(Wv, dtype=np.float32)
    ident, tribias = _consts()

    in_maps = []
    for c in range(N_CORES):
        in_maps.append({
            "x": x[c * B_SHARD:(c + 1) * B_SHARD],
            "Wq": Wq, "Wk": Wk, "Wv": Wv,
            "ident": ident, "tribias": tribias,
        })

    res = bass_utils.run_bass_kernel_spmd(
        nc, in_maps, core_ids=list(range(N_CORES)), trace=TRACE)
    LAST_RESULTS = res
    out = np.concatenate([res.results[c]["out"] for c in range(N_CORES)], axis=0)
    return out


# revision 10
# speedup vs baseline: 1.3320x; 1.1340x over previous
"""Trainium2 Bass kernel for a causal single-head attention block.

Reference computation (fp32):
    q = x @ Wq; k = x @ Wk; v = x @ Wv        x: [B=256, T=256, C=384], W*: [384, 64]
    wei = softmax(causal_mask(q @ k.T / sqrt(C)))
    out = wei @ v                              out: [256, 256, 64]

Strategy: pure data parallel over B across 8 NeuronCores (32 batches/core).

Per-batch pipeline (software-pipelined across batches so the PE never waits on
the softmax chain):
    xT   = PE-transpose(x_b)                   [C, T], 3 chunks [128, 256] (fp32r)
    qT|kT = W.T @ xT                           one [64, 512] psum (fp32r, N=256)
    v    = xT.T @ Wv                           [128, 64+64] (both t tiles, one psum)
    sT   = kT[:, s_tile].T @ qT                scores transposed [s, t], [128, 512] psum
    P    = exp((sT + tri_bias) * 1/sqrt(C))    tri mask only on diagonal 128x128 blocks;
                                               off-diagonal blocks exp'd straight from
                                               psum (never masked) or zeroed (always
                                               masked)
    oeT  = [v | 1].T @ P                       [65, 256]; row 64 = softmax denominator Z
    out  = PE-transpose(oeT) * (1/Z)           normalize on ScalarE, DMA out

Engine split: PE matmuls; DVE psum->sbuf operand copies + masks + reciprocal;
ScalarE (ACT) exp + v/ones copies + final normalize; fp32r (TF32-like) operands
everywhere for single-pass matmuls at 1 cyc/row (N>=256).
"""

import numpy as np

N_EMBED = 384
HEAD_SIZE = 64
H1 = HEAD_SIZE + 1
T = 256
B = 256
N_CORES = 8
B_SHARD = B // N_CORES  # 32
CC = N_EMBED // 128  # 3 contraction chunks
INV_SQRT_C = 1.0 / float(np.sqrt(N_EMBED))
MASK_NEG = -1.0e9

_CACHE = {}

# test.py can flip these before calling kernel()
TRACE = False
LAST_RESULTS = None
USE_FP32R = True


def _build_program():
    import concourse.bacc as bacc
    import concourse.mybir as mybir
    import concourse.tile as tile
    from concourse import bass

    f32 = mybir.dt.float32
    mdt = mybir.dt.float32r if USE_FP32R else f32
    ts = bass.ts
    Exp = mybir.ActivationFunctionType.Exp
    Copy = mybir.ActivationFunctionType.Copy

    nc = bacc.Bacc("TRN2", target_bir_lowering=False, debug=False,
                   enable_asserts=False)

    x_d = nc.dram_tensor("x", [B_SHARD, T, N_EMBED], f32, kind="ExternalInput")
    wq_d = nc.dram_tensor("Wq", [N_EMBED, HEAD_SIZE], f32, kind="ExternalInput")
    wk_d = nc.dram_tensor("Wk", [N_EMBED, HEAD_SIZE], f32, kind="ExternalInput")
    wv_d = nc.dram_tensor("Wv", [N_EMBED, HEAD_SIZE], f32, kind="ExternalInput")
    ident_d = nc.dram_tensor("ident", [128, 128], f32, kind="ExternalInput")
    tri_d = nc.dram_tensor("maskbias", [2, 128, T], f32, kind="ExternalInput")
    out_d = nc.dram_tensor("out", [B_SHARD, T, HEAD_SIZE], f32, kind="ExternalOutput")

    x_ap = x_d.ap().bitcast(mdt)
    out_ap = out_d.ap()

    with tile.TileContext(nc) as tc:
        with (
            tc.tile_pool(name="const", bufs=1) as cpool,
            tc.tile_pool(name="xin", bufs=6) as xin_pool,
            tc.tile_pool(name="xt", bufs=2) as xt_pool,
            tc.tile_pool(name="proj", bufs=4) as proj_pool,
            tc.tile_pool(name="soft", bufs=6) as soft_pool,
            tc.tile_pool(name="outp", bufs=8) as out_pool,
            tc.tile_pool(name="ps", bufs=7, space="PSUM") as ps_pool,
            tc.tile_pool(name="pshb", bufs=1, space="PSUM") as cpool_ps,
        ):
            # ---- constants ----
            ident = cpool.tile([128, 128], mdt, tag="ident")
            nc.sync.dma_start(ident[:], ident_d.ap().bitcast(mdt))
            wq_sb, wk_sb, wv_sb = [], [], []
            for cc in range(CC):
                for name, dram, lst in (("wq", wq_d, wq_sb), ("wk", wk_d, wk_sb),
                                        ("wv", wv_d, wv_sb)):
                    stage = cpool.tile([128, HEAD_SIZE], f32, tag=f"{name}s{cc}")
                    nc.sync.dma_start(stage[:], dram.ap()[ts(cc, 128), :])
                    t_ = cpool.tile([128, HEAD_SIZE], mdt, tag=f"{name}{cc}")
                    nc.vector.tensor_copy(t_[:], stage[:])
                    lst.append(t_)
            tri_sb = []
            for st in range(2):
                t_ = cpool.tile([128, T], f32, tag=f"tri{st}")
                nc.sync.dma_start(t_[:], tri_d.ap()[st, :, :])
                tri_sb.append(t_)
            ones_col = cpool.tile([128, 1], f32, tag="ones")
            nc.gpsimd.memset(ones_col[:], 1.0)
            hb_sb = cpool.tile([128, 32], mybir.dt.bfloat16, tag="hb")
            nc.gpsimd.memset(hb_sb[:], 1.0)
            hb_ps = cpool_ps.tile([32, 32], f32, tag="hbps")

            def heartbeat():
                # tiny bf16 matmul to keep the PE HAM activity monitor warm
                nc.tensor.matmul(hb_ps[:], hb_sb[:, :32], hb_sb[:, :32],
                                 start=True, stop=True)

            def load_x(b):
                tiles = []
                for tt in range(2):
                    t_ = xin_pool.tile([128, N_EMBED], mdt, tag="x_nat")
                    nc.sync.dma_start(t_[:], x_ap[b, ts(tt, 128), :])
                    tiles.append(t_)
                return tiles

            def transpose_x(x_nat):
                """-> one [128, 768] sbuf tile; chunk cc at [:, cc*256:(cc+1)*256]"""
                psA = ps_pool.tile([128, 512], mdt, tag="ps")   # cc 0,1
                psB = ps_pool.tile([128, 512], mdt, tag="ps")   # cc 2
                for cc in range(CC):
                    p = psA if cc < 2 else psB
                    off = (cc % 2) * 256
                    for tt in range(2):
                        nc.tensor.transpose(p[:, off + tt * 128:off + (tt + 1) * 128],
                                            x_nat[tt][:, ts(cc, 128)], ident[:])
                sb = xt_pool.tile([128, 3 * T], mdt, tag="xt")
                nc.vector.tensor_copy(sb[:, :512], psA[:])
                nc.vector.tensor_copy(sb[:, 512:], psB[:, :256])
                return sb

            def qkT_mm(xt):
                ps = ps_pool.tile([128, 512], f32, tag="ps")
                for cc in range(CC):
                    xc = xt[:, ts(cc, T)]
                    nc.tensor.matmul(ps[:HEAD_SIZE, :T], wq_sb[cc][:], xc,
                                     start=(cc == 0), stop=(cc == CC - 1))
                for cc in range(CC):
                    xc = xt[:, ts(cc, T)]
                    nc.tensor.matmul(ps[:HEAD_SIZE, T:], wk_sb[cc][:], xc,
                                     start=(cc == 0), stop=(cc == CC - 1))
                sb = proj_pool.tile([HEAD_SIZE, 512], mdt, tag="qk")
                nc.vector.tensor_copy(sb[:], ps[:HEAD_SIZE, :])
                return sb  # qT = [:, :256], kT = [:, 256:]

            def v_mm(xt):
                ps = ps_pool.tile([128, 512], f32, tag="ps")
                for tt in range(2):
                    for cc in range(CC):
                        nc.tensor.matmul(ps[:, tt * 128:tt * 128 + HEAD_SIZE],
                                         xt[:, cc * T + tt * 128: cc * T + (tt + 1) * 128],
                                         wv_sb[cc][:],
                                         start=(cc == 0), stop=(cc == CC - 1))
                v_ext = []
                for tt in range(2):
                    sb = proj_pool.tile([128, H1], mdt, tag="v_ext")
                    nc.vector.tensor_copy(sb[:, :HEAD_SIZE],
                                          ps[:, tt * 128:tt * 128 + HEAD_SIZE])
                    nc.vector.tensor_copy(sb[:, HEAD_SIZE:H1], ones_col[:])
                    v_ext.append(sb)
                return v_ext

            def scores_mm(qk):
                ps = ps_pool.tile([128, 512], f32, tag="ps")
                for st in range(2):
                    nc.tensor.matmul(ps[:, ts(st, T)], qk[:, T + st * 128:T + (st + 1) * 128],
                                     qk[:, :T], start=True, stop=True)
                return ps  # scoresT: s_tile st at [:, st*256:(st+1)*256]

            def softmax(sc_ps):
                """P tiles [128, 256] fp32r per s_tile, causal-masked, unnormalized."""
                p_sb = []
                for st in range(2):
                    m = soft_pool.tile([128, T], f32, tag=f"m{st}")
                    nc.vector.tensor_add(m[:], sc_ps[:, ts(st, T)], tri_sb[st][:])
                    p = soft_pool.tile([128, T], mdt, tag=f"p{st}")
                    nc.scalar.activation(p[:], m[:], Exp, scale=INV_SQRT_C)
                    p_sb.append(p)
                return p_sb

            def oe_mm(v_ext, p_sb):
                ps = ps_pool.tile([128, 512], f32, tag="ps")
                for st in range(2):
                    nc.tensor.matmul(ps[:H1, :T], v_ext[st][:], p_sb[st][:],
                                     start=(st == 0), stop=(st == 1))
                sb = out_pool.tile([H1 + 1, T], mdt, tag="oe")
                # row 65 is never written by the matmul and never read after the
                # transpose; copying 66 rows keeps partition bases aligned
                nc.vector.tensor_copy(sb[:], ps[:H1 + 1, :T])
                return sb

            def fin_mm(oe):
                ps = ps_pool.tile([128, 512], mdt, tag="ps")
                for tt in range(2):
                    nc.tensor.transpose(ps[:, tt * 128:tt * 128 + H1 + 1],
                                        oe[:, ts(tt, 128)], ident[:H1 + 1, :H1 + 1])
                return ps

            def norm_store(b, fin_ps):
                fps = fin_ps[:].bitcast(f32)
                for tt in range(2):
                    rz = out_pool.tile([128, 1], f32, tag="rz")
                    nc.vector.reciprocal(
                        rz[:], fps[:, tt * 128 + HEAD_SIZE: tt * 128 + H1])
                    o = out_pool.tile([128, HEAD_SIZE], f32, tag="o")
                    nc.scalar.activation(o[:], fps[:, tt * 128: tt * 128 + HEAD_SIZE],
                                         Copy, scale=rz[:])
                    nc.sync.dma_start(out_ap[b, ts(tt, 128), :], o[:])

            # ---- software-pipelined batch loop ----
            # PE stream per iteration: qkT(b) v(b) sc(b) T(b+1) oe(b) finT(b-1);
            # the DVE/ACT softmax chain of batch b runs under T(b+1), and
            # finT(b-1)'s oe-copy dependency is a full iteration old.
            x_nat = load_x(0)
            load_x_next = load_x(1)
            xt = transpose_x(x_nat)
            prev_oe = None
            for b in range(B_SHARD):
                qk = qkT_mm(xt)
                heartbeat()
                v_ext = v_mm(xt)
                sc_ps = scores_mm(qk)
                p_sb = softmax(sc_ps)
                heartbeat()
                if b + 1 < B_SHARD:
                    xt = transpose_x(load_x_next)
                if b + 2 < B_SHARD:
                    load_x_next = load_x(b + 2)
                oe_prev = prev_oe
                heartbeat()
                prev_oe = oe_mm(v_ext, p_sb)
                if oe_prev is not None:
                    norm_store(b - 1, fin_mm(oe_prev))
            norm_store(B_SHARD - 1, fin_mm(prev_oe))

    nc.compile()
    return nc


def _consts():
    ident = np.eye(128, dtype=np.float32)
    # maskbias[st][s_local, t] = MASK_NEG where global s > t (causal mask)
    s = np.arange(T)[:, None]
    t = np.arange(T)[None, :]
    full = np.where(s > t, np.float32(MASK_NEG), np.float32(0.0)).astype(np.float32)
    bias = np.stack([full[:128], full[128:]], axis=0)
    return ident, bias


def kernel(x, Wq, Wk, Wv):
    global LAST_RESULTS
    from concourse import bass_utils

    if "nc" not in _CACHE:
        _CACHE["nc"] = _build_program()
    nc = _CACHE["nc"]

    x = np.ascontiguousarray(x, dtype=np.float32)
    Wq = np.ascontiguousarray(Wq, dtype=np.float32)
    Wk = np.ascontiguousarray(Wk, dtype=np.float32)
    Wv = np.ascontiguousarray(Wv, dtype=np.float32)
    ident, bias = _consts()

    in_maps = []
    for c in range(N_CORES):
        in_maps.append({
            "x": x[c * B_SHARD:(c + 1) * B_SHARD],
            "Wq": Wq, "Wk": Wk, "Wv": Wv,
            "ident": ident, "maskbias": bias,
        })

    res = bass_utils.run_bass_kernel_spmd(
        nc, in_maps, core_ids=list(range(N_CORES)), trace=TRACE)
    LAST_RESULTS = res
    out = np.concatenate([res.results[c]["out"] for c in range(N_CORES)], axis=0)
    return out


# revision 11
# speedup vs baseline: 1.3381x; 1.0046x over previous
"""Trainium2 Bass kernel for a causal single-head attention block.

Reference computation (fp32):
    q = x @ Wq; k = x @ Wk; v = x @ Wv        x: [B=256, T=256, C=384], W*: [384, 64]
    wei = softmax(causal_mask(q @ k.T / sqrt(C)))
    out = wei @ v                              out: [256, 256, 64]

Strategy: pure data parallel over B across 8 NeuronCores (32 batches/core).

Per-batch pipeline (software-pipelined across batches so the PE never waits on
the softmax chain):
    xT   = PE-transpose(x_b)                   [C, T], 3 chunks [128, 256] (fp32r)
    qT|kT = W.T @ xT                           one [64, 512] psum (fp32r, N=256)
    v    = xT.T @ Wv                           [128, 64+64] (both t tiles, one psum)
    sT   = kT[:, s_tile].T @ qT                scores transposed [s, t], [128, 512] psum
    P    = exp((sT + tri_bias) * 1/sqrt(C))    tri mask only on diagonal 128x128 blocks;
                                               off-diagonal blocks exp'd straight from
                                               psum (never masked) or zeroed (always
                                               masked)
    oeT  = [v | 1].T @ P                       [65, 256]; row 64 = softmax denominator Z
    out  = PE-transpose(oeT) * (1/Z)           normalize on ScalarE, DMA out

Engine split: PE matmuls; DVE psum->sbuf operand copies + masks + reciprocal;
ScalarE (ACT) exp + v/ones copies + final normalize; fp32r (TF32-like) operands
everywhere for single-pass matmuls at 1 cyc/row (N>=256).
"""

import numpy as np

N_EMBED = 384
HEAD_SIZE = 64
H1 = HEAD_SIZE + 1
T = 256
B = 256
N_CORES = 8
B_SHARD = B // N_CORES  # 32
CC = N_EMBED // 128  # 3 contraction chunks
INV_SQRT_C = 1.0 / float(np.sqrt(N_EMBED))
MASK_NEG = -1.0e9

_CACHE = {}

# test.py can flip these before calling kernel()
TRACE = False
LAST_RESULTS = None
USE_FP32R = True


def _build_program():
    import concourse.bacc as bacc
    import concourse.mybir as mybir
    import concourse.tile as tile
    from concourse import bass

    f32 = mybir.dt.float32
    mdt = mybir.dt.float32r if USE_FP32R else f32
    ts = bass.ts
    Exp = mybir.ActivationFunctionType.Exp
    Copy = mybir.ActivationFunctionType.Copy

    nc = bacc.Bacc("TRN2", target_bir_lowering=False, debug=False,
                   enable_asserts=False)

    x_d = nc.dram_tensor("x", [B_SHARD, T, N_EMBED], f32, kind="ExternalInput")
    wq_d = nc.dram_tensor("Wq", [N_EMBED, HEAD_SIZE], f32, kind="ExternalInput")
    wk_d = nc.dram_tensor("Wk", [N_EMBED, HEAD_SIZE], f32, kind="ExternalInput")
    wv_d = nc.dram_tensor("Wv", [N_EMBED, HEAD_SIZE], f32, kind="ExternalInput")
    ident_d = nc.dram_tensor("ident", [128, 128], f32, kind="ExternalInput")
    tri_d = nc.dram_tensor("maskbias", [2, 128, T], f32, kind="ExternalInput")
    out_d = nc.dram_tensor("out", [B_SHARD, T, HEAD_SIZE], f32, kind="ExternalOutput")

    x_ap = x_d.ap().bitcast(mdt)
    out_ap = out_d.ap()

    with tile.TileContext(nc) as tc:
        with (
            tc.tile_pool(name="const", bufs=1) as cpool,
            tc.tile_pool(name="xin", bufs=6) as xin_pool,
            tc.tile_pool(name="xt", bufs=2) as xt_pool,
            tc.tile_pool(name="proj", bufs=4) as proj_pool,
            tc.tile_pool(name="soft", bufs=6) as soft_pool,
            tc.tile_pool(name="outp", bufs=8) as out_pool,
            tc.tile_pool(name="ps", bufs=7, space="PSUM") as ps_pool,
            tc.tile_pool(name="pshb", bufs=1, space="PSUM") as cpool_ps,
        ):
            # ---- constants ----
            ident = cpool.tile([128, 128], mdt, tag="ident")
            nc.sync.dma_start(ident[:], ident_d.ap().bitcast(mdt))
            wq_sb, wk_sb, wv_sb = [], [], []
            for cc in range(CC):
                for name, dram, lst in (("wq", wq_d, wq_sb), ("wk", wk_d, wk_sb),
                                        ("wv", wv_d, wv_sb)):
                    stage = cpool.tile([128, HEAD_SIZE], f32, tag=f"{name}s{cc}")
                    nc.sync.dma_start(stage[:], dram.ap()[ts(cc, 128), :])
                    t_ = cpool.tile([128, HEAD_SIZE], mdt, tag=f"{name}{cc}")
                    nc.vector.tensor_copy(t_[:], stage[:])
                    lst.append(t_)
            tri_sb = []
            for st in range(2):
                t_ = cpool.tile([128, T], f32, tag=f"tri{st}")
                nc.sync.dma_start(t_[:], tri_d.ap()[st, :, :])
                tri_sb.append(t_)
            ones_col = cpool.tile([128, 1], f32, tag="ones")
            nc.gpsimd.memset(ones_col[:], 1.0)
            hb_sb = cpool.tile([128, 512], mybir.dt.bfloat16, tag="hb")
            nc.gpsimd.memset(hb_sb[:], 1.0)
            hb_ps = cpool_ps.tile([32, 512], f32, tag="hbps")

            def heartbeat(n=1, rows=32):
                # bf16 matmuls to keep the PE HAM activity monitor warm
                for _ in range(n):
                    nc.tensor.matmul(hb_ps[:32, :rows], hb_sb[:, :32],
                                     hb_sb[:, :rows], start=True, stop=True)

            # dense ~2x 4096-cycle windows of bf16 streaming at kernel start
            heartbeat(n=20, rows=512)

            def load_x(b):
                tiles = []
                for tt in range(2):
                    t_ = xin_pool.tile([128, N_EMBED], mdt, tag="x_nat")
                    nc.sync.dma_start(t_[:], x_ap[b, ts(tt, 128), :])
                    tiles.append(t_)
                return tiles

            def transpose_x(x_nat):
                """-> one [128, 768] sbuf tile; chunk cc at [:, cc*256:(cc+1)*256]"""
                psA = ps_pool.tile([128, 512], mdt, tag="ps")   # cc 0,1
                psB = ps_pool.tile([128, 512], mdt, tag="ps")   # cc 2
                for cc in range(CC):
                    p = psA if cc < 2 else psB
                    off = (cc % 2) * 256
                    for tt in range(2):
                        nc.tensor.transpose(p[:, off + tt * 128:off + (tt + 1) * 128],
                                            x_nat[tt][:, ts(cc, 128)], ident[:])
                sb = xt_pool.tile([128, 3 * T], mdt, tag="xt")
                nc.vector.tensor_copy(sb[:, :512], psA[:])
                nc.vector.tensor_copy(sb[:, 512:], psB[:, :256])
                return sb

            def qkT_mm(xt):
                ps = ps_pool.tile([128, 512], f32, tag="ps")
                for cc in range(CC):
                    xc = xt[:, ts(cc, T)]
                    nc.tensor.matmul(ps[:HEAD_SIZE, :T], wq_sb[cc][:], xc,
                                     start=(cc == 0), stop=(cc == CC - 1))
                for cc in range(CC):
                    xc = xt[:, ts(cc, T)]
                    nc.tensor.matmul(ps[:HEAD_SIZE, T:], wk_sb[cc][:], xc,
                                     start=(cc == 0), stop=(cc == CC - 1))
                sb = proj_pool.tile([HEAD_SIZE, 512], mdt, tag="qk")
                nc.vector.tensor_copy(sb[:], ps[:HEAD_SIZE, :])
                return sb  # qT = [:, :256], kT = [:, 256:]

            def v_mm(xt):
                ps = ps_pool.tile([128, 512], f32, tag="ps")
                for tt in range(2):
                    for cc in range(CC):
                        nc.tensor.matmul(ps[:, tt * 128:tt * 128 + HEAD_SIZE],
                                         xt[:, cc * T + tt * 128: cc * T + (tt + 1) * 128],
                                         wv_sb[cc][:],
                                         start=(cc == 0), stop=(cc == CC - 1))
                v_ext = []
                for tt in range(2):
                    sb = proj_pool.tile([128, H1], mdt, tag="v_ext")
                    nc.vector.tensor_copy(sb[:, :HEAD_SIZE],
                                          ps[:, tt * 128:tt * 128 + HEAD_SIZE])
                    nc.vector.tensor_copy(sb[:, HEAD_SIZE:H1], ones_col[:])
                    v_ext.append(sb)
                return v_ext

            def scores_mm(qk):
                ps = ps_pool.tile([128, 512], f32, tag="ps")
                for st in range(2):
                    nc.tensor.matmul(ps[:, ts(st, T)], qk[:, T + st * 128:T + (st + 1) * 128],
                                     qk[:, :T], start=True, stop=True)
                return ps  # scoresT: s_tile st at [:, st*256:(st+1)*256]

            def softmax(sc_ps):
                """P tiles [128, 256] fp32r per s_tile, causal-masked, unnormalized."""
                p_sb = []
                for st in range(2):
                    m = soft_pool.tile([128, T], f32, tag=f"m{st}")
                    nc.vector.tensor_add(m[:], sc_ps[:, ts(st, T)], tri_sb[st][:])
                    p = soft_pool.tile([128, T], mdt, tag=f"p{st}")
                    nc.scalar.activation(p[:], m[:], Exp, scale=INV_SQRT_C)
                    p_sb.append(p)
                return p_sb

            def oe_mm(v_ext, p_sb):
                ps = ps_pool.tile([128, 512], f32, tag="ps")
                for st in range(2):
                    nc.tensor.matmul(ps[:H1, :T], v_ext[st][:], p_sb[st][:],
                                     start=(st == 0), stop=(st == 1))
                sb = out_pool.tile([H1 + 1, T], mdt, tag="oe")
                # row 65 is never written by the matmul and never read after the
                # transpose; copying 66 rows keeps partition bases aligned
                nc.vector.tensor_copy(sb[:], ps[:H1 + 1, :T])
                return sb

            def fin_mm(oe):
                ps = ps_pool.tile([128, 512], mdt, tag="ps")
                for tt in range(2):
                    nc.tensor.transpose(ps[:, tt * 128:tt * 128 + H1 + 1],
                                        oe[:, ts(tt, 128)], ident[:H1 + 1, :H1 + 1])
                return ps

            def norm_store(b, fin_ps):
                fps = fin_ps[:].bitcast(f32)
                for tt in range(2):
                    rz = out_pool.tile([128, 1], f32, tag="rz")
                    nc.vector.reciprocal(
                        rz[:], fps[:, tt * 128 + HEAD_SIZE: tt * 128 + H1])
                    o = out_pool.tile([128, HEAD_SIZE], f32, tag="o")
                    nc.scalar.activation(o[:], fps[:, tt * 128: tt * 128 + HEAD_SIZE],
                                         Copy, scale=rz[:])
                    nc.sync.dma_start(out_ap[b, ts(tt, 128), :], o[:])

            # ---- software-pipelined batch loop ----
            # PE stream per iteration: qkT(b) v(b) sc(b) T(b+1) oe(b) finT(b-1);
            # the DVE/ACT softmax chain of batch b runs under T(b+1), and
            # finT(b-1)'s oe-copy dependency is a full iteration old.
            x_nat = load_x(0)
            load_x_next = load_x(1)
            xt = transpose_x(x_nat)
            prev_oe = None
            for b in range(B_SHARD):
                qk = qkT_mm(xt)
                heartbeat()
                v_ext = v_mm(xt)
                sc_ps = scores_mm(qk)
                p_sb = softmax(sc_ps)
                heartbeat()
                if b + 1 < B_SHARD:
                    xt = transpose_x(load_x_next)
                if b + 2 < B_SHARD:
                    load_x_next = load_x(b + 2)
                oe_prev = prev_oe
                heartbeat()
                prev_oe = oe_mm(v_ext, p_sb)
                if oe_prev is not None:
                    norm_store(b - 1, fin_mm(oe_prev))
            norm_store(B_SHARD - 1, fin_mm(prev_oe))

    nc.compile()
    return nc


def _consts():
    ident = np.eye(128, dtype=np.float32)
    # maskbias[st][s_local, t] = MASK_NEG where global s > t (causal mask)
    s = np.arange(T)[:, None]
    t = np.arange(T)[None, :]
    full = np.where(s > t, np.float32(MASK_NEG), np.float32(0.0)).astype(np.float32)
    bias = np.stack([full[:128], full[128:]], axis=0)
    return ident, bias


def kernel(x, Wq, Wk, Wv):
    global LAST_RESULTS
    from concourse import bass_utils

    if "nc" not in _CACHE:
        _CACHE["nc"] = _build_program()
    nc = _CACHE["nc"]

    x = np.ascontiguousarray(x, dtype=np.float32)
    Wq = np.ascontiguousarray(Wq, dtype=np.float32)
    Wk = np.ascontiguousarray(Wk, dtype=np.float32)
    Wv = np.ascontiguousarray(Wv, dtype=np.float32)
    ident, bias = _consts()

    in_maps = []
    for c in range(N_CORES):
        in_maps.append({
            "x": x[c * B_SHARD:(c + 1) * B_SHARD],
            "Wq": Wq, "Wk": Wk, "Wv": Wv,
            "ident": ident, "maskbias": bias,
        })

    res = bass_utils.run_bass_kernel_spmd(
        nc, in_maps, core_ids=list(range(N_CORES)), trace=TRACE)
    LAST_RESULTS = res
    out = np.concatenate([res.results[c]["out"] for c in range(N_CORES)], axis=0)
    return out


# revision 13
# speedup vs baseline: 1.3655x; 1.0204x over previous
"""Trainium2 Bass kernel for a causal single-head attention block.

Reference computation (fp32):
    q = x @ Wq; k = x @ Wk; v = x @ Wv        x: [B=256, T=256, C=384], W*: [384, 64]
    wei = softmax(causal_mask(q @ k.T / sqrt(C)))
    out = wei @ v                              out: [256, 256, 64]

Strategy: pure data parallel over B across 8 NeuronCores (32 batches/core).

All matmul operands are fp16: 2-byte operands stream through the PE at full
rate (1 cyc/row; 4-byte fp32/fp32r streams at half rate and never registers as
HAM activity, pinning the PE clock at 1.2 GHz), get fast weight load, and keep
the clock-gate warm. PSUM accumulation stays fp32, so measured end-to-end error
vs the fp32 reference is ~7e-4 (scale-relative).

Per-batch pipeline (software-pipelined across batches so the PE never waits on
the softmax chain):
    xT   = PE-transpose(x_b)                   [C, T] fp16, one [128, 768] psum bank
    qT|kT = W.T @ xT                           one [64, 512] psum (N=256 matmuls)
    v    = xT.T @ Wv                           [128, 64+64] (both t tiles, one psum)
    sT   = kT[:, s_tile].T @ qT                scores transposed [s, t], [128, 512] psum
    P    = exp(sT * 1/sqrt(C)) * mask01        mask after exp: 0/1 multiply (GpSimd)
    oeT  = [v | 1].T @ P                       [65, 256] psum; row 64 = denominator Z
    out  = PE-transpose(oeT) * (1/Z)           normalize on ScalarE, DMA out

PE stream per iteration: qkT(b) v(b) sc(b) T(b+1) oe(b) finT(b-1); the DVE/ACT
softmax chain of batch b runs under T(b+1), and finT(b-1)'s oe-copy dependency
is a full iteration old.
"""

import numpy as np

N_EMBED = 384
HEAD_SIZE = 64
H1 = HEAD_SIZE + 1
T = 256
B = 256
N_CORES = 8
B_SHARD = B // N_CORES  # 32
CC = N_EMBED // 128  # 3 contraction chunks
INV_SQRT_C = 1.0 / float(np.sqrt(N_EMBED))

_CACHE = {}

# test.py can flip these before calling kernel()
TRACE = False
LAST_RESULTS = None


def _build_program():
    import concourse.bacc as bacc
    import concourse.mybir as mybir
    import concourse.tile as tile
    from concourse import bass

    f32 = mybir.dt.float32
    f16 = mybir.dt.float16
    ts = bass.ts
    Exp = mybir.ActivationFunctionType.Exp
    Copy = mybir.ActivationFunctionType.Copy

    nc = bacc.Bacc("TRN2", target_bir_lowering=False, debug=False,
                   enable_asserts=False)

    x_d = nc.dram_tensor("x", [B_SHARD, T, N_EMBED], f16, kind="ExternalInput")
    wq_d = nc.dram_tensor("Wq", [N_EMBED, HEAD_SIZE], f16, kind="ExternalInput")
    wk_d = nc.dram_tensor("Wk", [N_EMBED, HEAD_SIZE], f16, kind="ExternalInput")
    wv_d = nc.dram_tensor("Wv", [N_EMBED, HEAD_SIZE], f16, kind="ExternalInput")
    ident_d = nc.dram_tensor("ident", [128, 128], f16, kind="ExternalInput")
    mask_d = nc.dram_tensor("mask01", [2, 128, T], f16, kind="ExternalInput")
    out_d = nc.dram_tensor("out", [B_SHARD, T, HEAD_SIZE], f32, kind="ExternalOutput")

    x_ap = x_d.ap()
    out_ap = out_d.ap()

    with tile.TileContext(nc) as tc:
        with (
            tc.tile_pool(name="const", bufs=1) as cpool,
            tc.tile_pool(name="xin", bufs=6) as xin_pool,
            tc.tile_pool(name="xt", bufs=2) as xt_pool,
            tc.tile_pool(name="proj", bufs=4) as proj_pool,
            tc.tile_pool(name="soft", bufs=6) as soft_pool,
            tc.tile_pool(name="outp", bufs=8) as out_pool,
            tc.tile_pool(name="ps", bufs=8, space="PSUM") as ps_pool,
        ):
            # ---- constants ----
            ident = cpool.tile([128, 128], f16, tag="ident")
            nc.sync.dma_start(ident[:], ident_d.ap())
            wq_sb, wk_sb, wv_sb = [], [], []
            for cc in range(CC):
                for name, dram, lst in (("wq", wq_d, wq_sb), ("wk", wk_d, wk_sb),
                                        ("wv", wv_d, wv_sb)):
                    t_ = cpool.tile([128, HEAD_SIZE], f16, tag=f"{name}{cc}")
                    nc.sync.dma_start(t_[:], dram.ap()[ts(cc, 128), :])
                    lst.append(t_)
            mask_sb = []
            for st in range(2):
                t_ = cpool.tile([128, T], f16, tag=f"mask{st}")
                nc.sync.dma_start(t_[:], mask_d.ap()[st, :, :])
                mask_sb.append(t_)
            ones_col = cpool.tile([128, 1], f16, tag="ones")
            nc.gpsimd.memset(ones_col[:], 1.0)

            def load_x(b):
                tiles = []
                for tt in range(2):
                    t_ = xin_pool.tile([128, N_EMBED], f16, tag="x_nat")
                    nc.sync.dma_start(t_[:], x_ap[b, ts(tt, 128), :])
                    tiles.append(t_)
                return tiles

            def transpose_x(x_nat):
                """-> one [128, 768] fp16 sbuf tile; chunk cc at [:, cc*256:(cc+1)*256]"""
                ps = ps_pool.tile([128, 3 * T], f16, tag="ps")
                for cc in range(CC):
                    for tt in range(2):
                        nc.tensor.transpose(
                            ps[:, cc * T + tt * 128: cc * T + (tt + 1) * 128],
                            x_nat[tt][:, ts(cc, 128)], ident[:])
                sb = xt_pool.tile([128, 3 * T], f16, tag="xt")
                nc.vector.tensor_copy(sb[:], ps[:])
                return sb

            def qkT_mm(xt):
                ps = ps_pool.tile([128, 512], f32, tag="ps")
                for cc in range(CC):
                    nc.tensor.matmul(ps[:HEAD_SIZE, :T], wq_sb[cc][:],
                                     xt[:, ts(cc, T)],
                                     start=(cc == 0), stop=(cc == CC - 1))
                for cc in range(CC):
                    nc.tensor.matmul(ps[:HEAD_SIZE, T:], wk_sb[cc][:],
                                     xt[:, ts(cc, T)],
                                     start=(cc == 0), stop=(cc == CC - 1))
                sb = proj_pool.tile([HEAD_SIZE, 512], f16, tag="qk")
                nc.vector.tensor_copy(sb[:], ps[:HEAD_SIZE, :])
                return sb  # qT = [:, :256], kT = [:, 256:]

            def v_mm(xt):
                ps = ps_pool.tile([128, 512], f32, tag="ps")
                for tt in range(2):
                    for cc in range(CC):
                        nc.tensor.matmul(ps[:, tt * 128:tt * 128 + HEAD_SIZE],
                                         xt[:, cc * T + tt * 128: cc * T + (tt + 1) * 128],
                                         wv_sb[cc][:],
                                         start=(cc == 0), stop=(cc == CC - 1))
                v_ext = []
                for tt in range(2):
                    sb = proj_pool.tile([128, H1], f16, tag="v_ext")
                    nc.vector.tensor_copy(sb[:, :HEAD_SIZE],
                                          ps[:, tt * 128:tt * 128 + HEAD_SIZE])
                    nc.vector.tensor_copy(sb[:, HEAD_SIZE:H1], ones_col[:])
                    v_ext.append(sb)
                return v_ext

            def scores_mm(qk):
                ps = ps_pool.tile([128, 512], f32, tag="ps")
                for st in range(2):
                    nc.tensor.matmul(ps[:, ts(st, T)],
                                     qk[:, T + st * 128:T + (st + 1) * 128],
                                     qk[:, :T], start=True, stop=True)
                return ps  # scoresT: s_tile st at [:, st*256:(st+1)*256]

            def softmax(sc_ps):
                """P [128, 256] fp16 per s_tile: exp then 0/1-mask, unnormalized."""
                p_sb = []
                for st in range(2):
                    e = soft_pool.tile([128, T], f16, tag=f"e{st}")
                    nc.scalar.activation(e[:], sc_ps[:, ts(st, T)], Exp,
                                         scale=INV_SQRT_C)
                    p = soft_pool.tile([128, T], f16, tag=f"p{st}")
                    nc.gpsimd.tensor_mul(p[:], e[:], mask_sb[st][:])
                    p_sb.append(p)
                return p_sb

            def oe_mm(v_ext, p_sb):
                ps = ps_pool.tile([128, 512], f32, tag="ps")
                for st in range(2):
                    nc.tensor.matmul(ps[:H1, :T], v_ext[st][:], p_sb[st][:],
                                     start=(st == 0), stop=(st == 1))
                sb = out_pool.tile([H1 + 1, T], f16, tag="oe")
                # row 65 is never written by the matmul and never read after the
                # transpose; copying 66 rows keeps partition bases aligned
                nc.vector.tensor_copy(sb[:], ps[:H1 + 1, :T])
                return sb

            def fin_mm(oe):
                ps = ps_pool.tile([128, 512], f16, tag="ps")
                for tt in range(2):
                    nc.tensor.transpose(ps[:, tt * 128:tt * 128 + H1 + 1],
                                        oe[:, ts(tt, 128)],
                                        ident[:H1 + 1, :H1 + 1])
                return ps

            def norm_store(b, fin_ps):
                fps = fin_ps[:]
                rz = out_pool.tile([128, 2], f32, tag="rz")
                # both Z columns (offsets 64 and 192) in one strided reciprocal
                nc.vector.reciprocal(rz[:], fps[:, HEAD_SIZE:256:128])
                for tt in range(2):
                    o = out_pool.tile([128, HEAD_SIZE], f32, tag="o")
                    nc.scalar.activation(o[:], fps[:, tt * 128: tt * 128 + HEAD_SIZE],
                                         Copy, scale=rz[:, tt:tt + 1])
                    nc.sync.dma_start(out_ap[b, ts(tt, 128), :], o[:])

            # ---- software-pipelined batch loop ----
            x_nat = load_x(0)
            load_x_next = load_x(1)
            xt = transpose_x(x_nat)
            prev_oe = None
            for b in range(B_SHARD):
                qk = qkT_mm(xt)
                v_ext = v_mm(xt)
                sc_ps = scores_mm(qk)
                p_sb = softmax(sc_ps)
                if b + 1 < B_SHARD:
                    xt = transpose_x(load_x_next)
                if b + 2 < B_SHARD:
                    load_x_next = load_x(b + 2)
                oe_prev = prev_oe
                prev_oe = oe_mm(v_ext, p_sb)
                if oe_prev is not None:
                    norm_store(b - 1, fin_mm(oe_prev))
            norm_store(B_SHARD - 1, fin_mm(prev_oe))

    nc.compile()
    return nc


def _consts():
    ident = np.eye(128, dtype=np.float16)
    # mask01[st][s_local, t] = 0 where global s > t (causal), else 1
    s = np.arange(T)[:, None]
    t = np.arange(T)[None, :]
    full = (s <= t).astype(np.float16)
    mask01 = np.stack([full[:128], full[128:]], axis=0)
    return ident, mask01


def kernel(x, Wq, Wk, Wv):
    global LAST_RESULTS
    from concourse import bass_utils

    if "nc" not in _CACHE:
        _CACHE["nc"] = _build_program()
    nc = _CACHE["nc"]

    x16 = np.ascontiguousarray(x, dtype=np.float16)
    Wq16 = np.ascontiguousarray(Wq, dtype=np.float16)
    Wk16 = np.ascontiguousarray(Wk, dtype=np.float16)
    Wv16 = np.ascontiguousarray(Wv, dtype=np.float16)
    ident, mask01 = _consts()

    in_maps = []
    for c in range(N_CORES):
        in_maps.append({
            "x": x16[c * B_SHARD:(c + 1) * B_SHARD],
            "Wq": Wq16, "Wk": Wk16, "Wv": Wv16,
            "ident": ident, "mask01": mask01,
        })

    res = bass_utils.run_bass_kernel_spmd(
        nc, in_maps, core_ids=list(range(N_CORES)), trace=TRACE)
    LAST_RESULTS = res
    out = np.concatenate([res.results[c]["out"] for c in range(N_CORES)], axis=0)
    return np.ascontiguousarray(out, dtype=np.float32)
